# revision 2
# baseline (speedup 1.0000x reference)
"""GCN forward on 8 TRN2 NeuronCores — regularized schedule + For_i-compressed
one-hot-matmul aggregation.

Math (reference.py):
  dinv = (indeg+1)^-1/2
  table1[s] = dinv[s]*(x@W1)[s]          (bf16, AllGathered per 1792-row slab)
  aggT1[f,t] = sum_e table1[row_e]^T one-hot  + dinv[t]^2*table1loc[t]^T (self)
  y1T = relu(aggT1*dinvT + b1)  ; t2preT = y1T*dinvT
  table2[t] = (t2preT^T @ W2)[t]          (f32 64-dim, AllGathered)
  aggT2[o,t] = sum_e table2[row_e]^T one-hot + self
  yT = aggT2*dinvT + b2

Schedule: targets split into 7 supergroups (sg) x 7 windows of 256; sources
into 7 chunks (each core's s-th 1792-row slab, concat across cores = 14336
rows, int16-addressable). Every (sg, chunk, window) cell is padded to a
uniform BPC blocks of 128 edges, so the aggregation is affine in sg and runs
as a hardware For_i loop. Self-loops are applied as per-group diagonal
matmuls (local slab x diag(dinv^2) one-hot), keeping BPC low.
"""

import sys

sys.path.insert(0, "/opt/trn_rl_repo")
import numpy as np
import ml_dtypes

import concourse.bass as bass
import concourse.mybir as mybir
import concourse.tile as tile
from concourse import bacc
from concourse.bass import ds
from concourse.bass_utils import run_bass_kernel_spmd

F32 = mybir.dt.float32
BF16 = mybir.dt.bfloat16
I16 = mybir.dt.int16
AF = mybir.ActivationFunctionType
ALU = mybir.AluOpType

P = 128
N, E = 100000, 1600000
IN, HID, OUT = 128, 128, 64
NCORES = 8
TPC = 12500
PPC = 12544
NG = PPC // P          # 98 groups of 128 nodes per core
NSG = 7                # supergroups per core
SGG = NG // NSG        # 14 groups per supergroup
SEG = SGG * P          # 1792 rows per (core, sg)
NCH = 7                # source chunks (one per sg-slab)
CHROWS = NCORES * SEG  # 14336 rows per gather chunk
NW = 7                 # target windows per supergroup
W = SEG // NW          # 256 targets per window
SENTINEL = 300.0       # colv value that never matches iota 0..255


def host_prep(edge_index):
    row = np.asarray(edge_index[0], dtype=np.int64)
    col = np.asarray(edge_index[1], dtype=np.int64)
    deg = np.bincount(col, minlength=N).astype(np.float64) + 1.0
    dinv_n = (1.0 / np.sqrt(deg)).astype(np.float32)

    n_all = np.arange(N, dtype=np.int64)
    gids = (n_all // TPC) * PPC + (n_all % TPC)
    grow = gids[row]
    gcol = gids[col]

    c_e = gcol // PPC
    lt = gcol - c_e * PPC
    s_e = lt // SEG
    r_e = lt % SEG
    w_e = r_e // W
    colw = (r_e % W).astype(np.float32)

    o_e = grow // PPC
    lsrc = grow - o_e * PPC
    ch_e = lsrc // SEG
    lr = (o_e * SEG + (lsrc % SEG)).astype(np.int16)

    cell = ((s_e * NCH + ch_e) * NW + w_e).astype(np.int64)  # 343 cells
    NCELL = NSG * NCH * NW
    counts = np.zeros((NCORES, NCELL), np.int64)
    percore = []
    for c in range(NCORES):
        sel = c_e == c
        cc = cell[sel]
        order = np.argsort(cc, kind="stable")
        counts[c] = np.bincount(cc, minlength=NCELL)
        percore.append((cc[order], lr[sel][order], colw[sel][order]))

    BPC = int(-(-counts.max() // P))  # uniform blocks per cell
    TOTB = NCELL * BPC

    gi_list, colv_list = [], []
    for c in range(NCORES):
        cc, lrc, cwc = percore[c]
        cell_start = np.zeros(NCELL, np.int64)
        cell_start[1:] = np.cumsum(counts[c])[:-1]
        rank = np.arange(cc.shape[0]) - cell_start[cc]
        dest = cc * (BPC * P) + rank
        gi_arr = np.zeros(TOTB * P, np.int16)
        cv_arr = np.full(TOTB * P, SENTINEL, np.float32)
        gi_arr[dest] = lrc
        cv_arr[dest] = cwc
        gi_list.append(np.ascontiguousarray(gi_arr.reshape(-1, 16).T))
        colv_list.append(
            np.ascontiguousarray(
                cv_arr.reshape(TOTB, P).T.astype(ml_dtypes.bfloat16)
            )
        )

    # per-node dinv in several layouts (padded nodes get 1.0)
    dinvP_list, dinv2P_list, dinvrow_list = [], [], []
    for c in range(NCORES):
        dloc = np.ones(PPC, np.float32)
        dloc[:TPC] = dinv_n[c * TPC : (c + 1) * TPC]
        dinvrow_list.append(dloc[None, :].copy())
        dinvP_list.append(np.ascontiguousarray(dloc.reshape(NG, P).T))
        dinv2P_list.append(np.ascontiguousarray((dloc * dloc).reshape(NG, P).T))

    sched = {"BPC": BPC, "TOTB": TOTB}
    return sched, gi_list, colv_list, dinvP_list, dinv2P_list, dinvrow_list


def build_kernel(sched, stage=9, debug=None):
    BPC = sched["BPC"]
    TOTB = sched["TOTB"]
    BSC = NW * BPC          # blocks per (sg, chunk) gather call
    NIDX = BSC * P          # idxs per gather call

    nc = bacc.Bacc("TRN2", target_bir_lowering=False, num_devices=NCORES)
    xTb = nc.dram_tensor("xTb", [P, PPC], BF16, kind="ExternalInput")
    gi = nc.dram_tensor("gi", [16, TOTB * 8], I16, kind="ExternalInput")
    colv = nc.dram_tensor("colv", [P, TOTB], BF16, kind="ExternalInput")
    dinvP = nc.dram_tensor("dinvP", [P, NG], F32, kind="ExternalInput")
    dinv2P = nc.dram_tensor("dinv2P", [P, NG], F32, kind="ExternalInput")
    dinvrow = nc.dram_tensor("dinvrow", [1, PPC], F32, kind="ExternalInput")
    W1 = nc.dram_tensor("W1", [IN, HID], F32, kind="ExternalInput")
    W2 = nc.dram_tensor("W2", [HID, OUT], F32, kind="ExternalInput")
    b1c = nc.dram_tensor("b1c", [P, 1], F32, kind="ExternalInput")
    b2c = nc.dram_tensor("b2c", [P, 1], F32, kind="ExternalInput")
    iota = nc.dram_tensor("iota", [P, W], BF16, kind="ExternalInput")
    iotaPc = nc.dram_tensor("iotaPc", [P, 2], F32, kind="ExternalInput")
    y = nc.dram_tensor("y", [OUT, PPC], F32, kind="ExternalOutput")
    dbg = None
    if debug is not None:
        dbg = nc.dram_tensor("dbg", [PPC, HID], F32, kind="ExternalOutput")

    t1q = nc.dram_tensor("t1q", [PPC, HID], BF16)
    t2q = nc.dram_tensor("t2q", [PPC, OUT], F32)
    table1 = nc.dram_tensor("table1", [NCH, CHROWS, HID], BF16, addr_space="Shared")
    table2 = nc.dram_tensor("table2", [NCH, CHROWS, OUT], F32, addr_space="Shared")

    t1qr = t1q.rearrange("(n p) f -> p n f", p=P)
    t2qr = t2q.rearrange("(n p) f -> p n f", p=P)

    with tile.TileContext(nc) as tc:
        with (
            tc.tile_pool(name="const", bufs=1) as cpool,
            tc.tile_pool(name="sb", bufs=2) as sb,
            tc.tile_pool(name="ps", bufs=1, space="PSUM") as psp,
        ):
            # ---- constants ----
            W1b = cpool.tile([IN, HID], BF16)
            nc.gpsimd.dma_start(out=W1b[:], in_=W1[:])
            W2b = cpool.tile([HID, OUT], BF16)
            nc.gpsimd.dma_start(out=W2b[:], in_=W2[:])
            b1t = cpool.tile([P, 1], F32)
            nc.sync.dma_start(out=b1t[:], in_=b1c[:])
            b2t = cpool.tile([P, 1], F32)
            nc.sync.dma_start(out=b2t[:], in_=b2c[:])
            iota_t = cpool.tile([P, W], BF16)
            nc.sync.dma_start(out=iota_t[:], in_=iota[:])
            iotaP_t = cpool.tile([P, 2], F32)
            nc.sync.dma_start(out=iotaP_t[:], in_=iotaPc[:])
            dinvP_t = cpool.tile([P, NG], F32)
            nc.sync.dma_start(out=dinvP_t[:], in_=dinvP[:])
            dinv2P_t = cpool.tile([P, NG], F32)
            nc.sync.dma_start(out=dinv2P_t[:], in_=dinv2P[:])
            gi_t = cpool.tile([P, TOTB * 8], I16)
            for r in range(8):
                nc.sync.dma_start(out=gi_t[16 * r : 16 * (r + 1), :], in_=gi[:])

            # ---- phase A: table1 rows = dinv[s] * (x @ W1), bf16 ----
            if stage >= 1:
                with tc.For_i(0, NSG, 1) as s:
                    xs = sb.tile([P, SEG], BF16, tag="msgs", name="xs")
                    nc.sync.dma_start(out=xs[:], in_=xTb[:, ds(s * SEG, SEG)])
                    dseg = sb.tile([P, SGG], F32, tag="dseg", name="dseg")
                    nc.sync.dma_start(out=dseg[:], in_=dinvP[:, ds(s * SGG, SGG)])
                    psA = psp.tile([P, SGG * P], F32, tag="psT", name="psA")
                    for gl in range(SGG):
                        nc.tensor.matmul(
                            out=psA[:, gl * P : (gl + 1) * P],
                            lhsT=xs[:, gl * P : (gl + 1) * P],
                            rhs=W1b[:],
                            start=True,
                            stop=True,
                        )
                    t1s = sb.tile([P, SGG, HID], BF16, tag="slab", name="t1s")
                    for gl in range(SGG):
                        nc.scalar.activation(
                            t1s[:, gl, :],
                            psA[:, gl * P : (gl + 1) * P],
                            AF.Copy,
                            scale=dseg[:, gl : gl + 1],
                        )
                    nc.sync.dma_start(
                        out=t1qr[:, ds(s * SGG, SGG), :], in_=t1s[:]
                    )
            if debug == "t1q":
                dq = cpool.tile([P, NG, HID], F32)
                nc.gpsimd.dma_start(out=dq[:], in_=t1qr[:])
                nc.sync.dma_start(
                    out=dbg.rearrange("(n p) f -> p n f", p=P)[:], in_=dq[:]
                )
            if stage >= 2:
                for s in range(NSG):
                    nc.gpsimd.collective_compute(
                        "AllGather",
                        ALU.bypass,
                        ins=[t1q[s * SEG : (s + 1) * SEG, :]],
                        outs=[table1[s]],
                        replica_groups=[list(range(NCORES))],
                    )

            def agg_layer(tablex, feat, msg_dtype, out_parts, lhs_slab, epilogue):
                """One aggregation layer inside a For_i over supergroups."""
                with tc.For_i(0, NSG, 1) as s:
                    dinvT = sb.tile([P, SEG], F32, tag="dinvT", name="dinvT", bufs=1)
                    nc.sync.dma_start(
                        out=dinvT[:],
                        in_=dinvrow[0:1, ds(s * SEG, SEG)].to_broadcast([P, SEG]),
                    )
                    psT = psp.tile([P, SEG], F32, tag="psT", name="psT")
                    nc.vector.memset(psT[:], 0.0)
                    for ch in range(NCH):
                        git = sb.tile([P, BSC * 8], I16, tag="git", name="git")
                        nc.sync.dma_start(
                            out=git[:],
                            in_=gi_t[:, ds((s * NCH + ch) * BSC * 8, BSC * 8)],
                        )
                        msgs = sb.tile([P, BSC, feat], msg_dtype, tag="msgs", name="msgs")
                        nc.gpsimd.dma_gather(
                            msgs[:],
                            tablex[ch],
                            git[:],
                            NIDX,
                            NIDX,
                            feat,
                            single_packet=False,
                        )
                        if msg_dtype != BF16:
                            mb = sb.tile([P, BSC, feat], BF16, tag="m2b", name="m2b")
                            nc.vector.tensor_copy(out=mb[:], in_=msgs[:])
                        else:
                            mb = msgs
                        cv = sb.tile([P, BSC], BF16, tag="cv", name="cv")
                        nc.sync.dma_start(
                            out=cv[:], in_=colv[:, ds((s * NCH + ch) * BSC, BSC)]
                        )
                        for w in range(NW):
                            S = sb.tile([P, BPC, W], BF16, tag="S", name="S")
                            nc.vector.tensor_tensor(
                                out=S[:],
                                in0=cv[:, w * BPC : (w + 1) * BPC, None].to_broadcast(
                                    [P, BPC, W]
                                ),
                                in1=iota_t[:, None, :].to_broadcast([P, BPC, W]),
                                op=ALU.is_equal,
                            )
                            for k in range(BPC):
                                nc.tensor.matmul(
                                    out=psT[0:out_parts, w * W : (w + 1) * W],
                                    lhsT=mb[:, w * BPC + k, :],
                                    rhs=S[:, k, :],
                                    start=False,
                                    stop=False,
                                    skip_group_check=True,
                                )
                    # self-loops: psT[:, gl-half] += lhsT=slab_gl, rhs=diag(dinv^2)
                    slab = lhs_slab(s)
                    for gl in range(SGG):
                        Sd = sb.tile([P, W], BF16, tag="Sd", name="Sd")
                        nc.vector.tensor_scalar(
                            out=Sd[:],
                            in0=iota_t[:],
                            scalar1=iotaP_t[:, (gl % 2) : (gl % 2) + 1],
                            scalar2=None,
                            op0=ALU.is_equal,
                        )
                        nc.tensor.matmul(
                            out=psT[0:out_parts, (gl // 2) * W : (gl // 2 + 1) * W],
                            lhsT=slab[:, gl, :],
                            rhs=Sd[:],
                            start=False,
                            stop=(gl % 2 == 1),
                            skip_group_check=True,
                        )
                    epilogue(s, psT, dinvT)

            # ---- L1 ----
            def lhs_slab1(s):
                t1slab = sb.tile([P, SGG, HID], BF16, tag="slab", name="t1slab")
                nc.sync.dma_start(out=t1slab[:], in_=t1qr[:, ds(s * SGG, SGG), :])
                return t1slab

            def epi1(s, psT, dinvT):
                tmpT = sb.tile([P, SEG], F32, tag="tmpT", name="tmpT", bufs=1)
                nc.vector.tensor_tensor(
                    out=tmpT[:], in0=psT[:], in1=dinvT[:], op=ALU.mult
                )
                y1T = sb.tile([P, SEG], F32, tag="y1T", name="y1T", bufs=1)
                nc.vector.tensor_scalar(
                    out=y1T[:],
                    in0=tmpT[:],
                    scalar1=b1t[:],
                    scalar2=0.0,
                    op0=ALU.add,
                    op1=ALU.max,
                )
                t2preT = sb.tile([P, SEG], BF16, tag="t2preT", name="t2preT", bufs=1)
                nc.vector.tensor_tensor(
                    out=t2preT[:], in0=y1T[:], in1=dinvT[:], op=ALU.mult
                )
                ps2 = psp.tile([P, SGG * OUT], F32, tag="ps2", name="ps2")
                for gl in range(SGG):
                    nc.tensor.matmul(
                        out=ps2[:, gl * OUT : (gl + 1) * OUT],
                        lhsT=t2preT[:, gl * P : (gl + 1) * P],
                        rhs=W2b[:],
                        start=True,
                        stop=True,
                    )
                t2s = sb.tile([P, SGG, OUT], F32, tag="t2s", name="t2s")
                nc.scalar.activation(t2s[:], ps2[:], AF.Copy)
                nc.sync.dma_start(out=t2qr[:, ds(s * SGG, SGG), :], in_=t2s[:])

            if stage >= 3:
                agg_layer(table1, HID, BF16, P, lhs_slab1, epi1)

            if debug == "t2q":
                dq2 = cpool.tile([P, NG, OUT], F32)
                nc.sync.dma_start(out=dq2[:], in_=t2qr[:])
                nc.sync.dma_start(
                    out=dbg.rearrange("(n p) f -> p n f", p=P)[:, :, :OUT], in_=dq2[:]
                )
            if debug == "dinvT":
                dv = sb.tile([P, SEG], F32, tag="dinvT", name="dvdump", bufs=1)
                nc.sync.dma_start(
                    out=dv[:], in_=dinvrow[0:1, 0:SEG].to_broadcast([P, SEG])
                )
                nc.sync.dma_start(
                    out=dbg[0:SEG, 0:P].rearrange("a b -> b a"), in_=dv[:]
                )
            if stage >= 4:
                for s in range(NSG):
                    nc.gpsimd.collective_compute(
                        "AllGather",
                        ALU.bypass,
                        ins=[t2q[s * SEG : (s + 1) * SEG, :]],
                        outs=[table2[s]],
                        replica_groups=[list(range(NCORES))],
                    )

            # ---- L2 ----
            def lhs_slab2(s):
                t2slabf = sb.tile([P, SGG, OUT], F32, tag="t2s", name="t2slabf")
                nc.sync.dma_start(out=t2slabf[:], in_=t2qr[:, ds(s * SGG, SGG), :])
                t2slab = sb.tile([P, SGG, OUT], BF16, tag="slab", name="t2slab")
                nc.vector.tensor_copy(out=t2slab[:], in_=t2slabf[:])
                return t2slab

            def epi2(s, psT, dinvT):
                outT = sb.tile([OUT, SEG], F32, tag="outT", name="outT", bufs=1)
                nc.vector.tensor_tensor(
                    out=outT[:], in0=psT[0:OUT, :], in1=dinvT[0:OUT, :], op=ALU.mult
                )
                outT2 = sb.tile([OUT, SEG], F32, tag="outT2", name="outT2", bufs=1)
                nc.vector.tensor_scalar(
                    out=outT2[:],
                    in0=outT[:],
                    scalar1=b2t[0:OUT, :],
                    scalar2=None,
                    op0=ALU.add,
                )
                nc.sync.dma_start(out=y[:, ds(s * SEG, SEG)], in_=outT2[:])

            if stage >= 5:
                agg_layer(table2, OUT, F32, OUT, lhs_slab2, epi2)

    nc.finalize()
    return nc


def make_in_maps(inputs, sched, gi_list, colv_list, dinvP_list, dinv2P_list, dinvrow_list):
    x = np.asarray(inputs["x"], np.float32)
    W1 = np.asarray(inputs["W1"], np.float32)
    W2 = np.asarray(inputs["W2"], np.float32)
    b1 = np.asarray(inputs["b1"], np.float32)
    b2 = np.asarray(inputs["b2"], np.float32)
    iota_np = np.tile(np.arange(W, dtype=ml_dtypes.bfloat16)[None, :], (P, 1))
    iotaPc_np = np.zeros((P, 2), np.float32)
    iotaPc_np[:, 0] = np.arange(P, dtype=np.float32)
    iotaPc_np[:, 1] = (np.arange(P) + 128).astype(np.float32)
    b1col = b1[:, None].astype(np.float32)
    b2col = np.zeros((P, 1), np.float32)
    b2col[:OUT, 0] = b2
    in_maps = []
    for c in range(NCORES):
        xs = np.zeros((P, PPC), ml_dtypes.bfloat16)
        xs[:, :TPC] = x[c * TPC : (c + 1) * TPC].T.astype(ml_dtypes.bfloat16)
        in_maps.append(
            {
                "xTb": xs,
                "gi": gi_list[c],
                "colv": colv_list[c],
                "dinvP": dinvP_list[c],
                "dinv2P": dinv2P_list[c],
                "dinvrow": dinvrow_list[c],
                "W1": W1,
                "W2": W2,
                "b1c": b1col,
                "b2c": b2col,
                "iota": iota_np,
                "iotaPc": iotaPc_np,
            }
        )
    return in_maps


def assemble_output(results):
    outs = []
    for c in range(NCORES):
        yc = results[c]["y"]  # [OUT, PPC]
        outs.append(yc[:, :TPC].T)
    return np.ascontiguousarray(np.concatenate(outs, axis=0))


def kernel(**inputs):
    prep = host_prep(inputs["edge_index"])
    sched = prep[0]
    nc = build_kernel(sched)
    in_maps = make_in_maps(inputs, *prep)
    res = run_bass_kernel_spmd(nc, in_maps, core_ids=list(range(NCORES)))
    return assemble_output(res.results)


# revision 3
# speedup vs baseline: 1.3339x; 1.3339x over previous
"""GCN forward on 8 TRN2 NeuronCores — regularized schedule + For_i-compressed
one-hot-matmul aggregation.

Math (reference.py):
  dinv = (indeg+1)^-1/2
  table1[s] = dinv[s]*(x@W1)[s]          (bf16, AllGathered per 1792-row slab)
  aggT1[f,t] = sum_e table1[row_e]^T one-hot  + dinv[t]^2*table1loc[t]^T (self)
  y1T = relu(aggT1*dinvT + b1)  ; t2preT = y1T*dinvT
  table2[t] = (t2preT^T @ W2)[t]          (f32 64-dim, AllGathered)
  aggT2[o,t] = sum_e table2[row_e]^T one-hot + self
  yT = aggT2*dinvT + b2

Schedule: targets split into 7 supergroups (sg) x 7 windows of 256; sources
into 7 chunks (each core's s-th 1792-row slab, concat across cores = 14336
rows, int16-addressable). Every (sg, chunk, window) cell is padded to a
uniform BPC blocks of 128 edges, so the aggregation is affine in sg and runs
as a hardware For_i loop. Self-loops are applied as per-group diagonal
matmuls (local slab x diag(dinv^2) one-hot), keeping BPC low.
"""

import sys

sys.path.insert(0, "/opt/trn_rl_repo")
import numpy as np
import ml_dtypes

import concourse.bass as bass
import concourse.mybir as mybir
import concourse.tile as tile
from concourse import bacc
from concourse.bass import ds
from concourse.bass_utils import run_bass_kernel_spmd

F32 = mybir.dt.float32
BF16 = mybir.dt.bfloat16
I16 = mybir.dt.int16
AF = mybir.ActivationFunctionType
ALU = mybir.AluOpType

P = 128
N, E = 100000, 1600000
IN, HID, OUT = 128, 128, 64
NCORES = 8
TPC = 12500
PPC = 12544
NG = PPC // P          # 98 groups of 128 nodes per core
NSG = 7                # supergroups per core
SGG = NG // NSG        # 14 groups per supergroup
SEG = SGG * P          # 1792 rows per (core, sg)
NCH = 7                # source chunks (one per sg-slab)
CHROWS = NCORES * SEG  # 14336 rows per gather chunk
NW = 7                 # target windows per supergroup
W = SEG // NW          # 256 targets per window
SENTINEL = 300.0       # colv value that never matches iota 0..255


def host_prep(edge_index):
    row = np.asarray(edge_index[0], dtype=np.int64)
    col = np.asarray(edge_index[1], dtype=np.int64)
    deg = np.bincount(col, minlength=N).astype(np.float64) + 1.0
    dinv_n = (1.0 / np.sqrt(deg)).astype(np.float32)

    n_all = np.arange(N, dtype=np.int64)
    gids = (n_all // TPC) * PPC + (n_all % TPC)
    grow = gids[row]
    gcol = gids[col]

    c_e = gcol // PPC
    lt = gcol - c_e * PPC
    s_e = lt // SEG
    r_e = lt % SEG
    w_e = r_e // W
    colw = (r_e % W).astype(np.float32)

    o_e = grow // PPC
    lsrc = grow - o_e * PPC
    ch_e = lsrc // SEG
    lr = (o_e * SEG + (lsrc % SEG)).astype(np.int16)

    cell = ((s_e * NCH + ch_e) * NW + w_e).astype(np.int64)  # 343 cells
    NCELL = NSG * NCH * NW
    counts = np.zeros((NCORES, NCELL), np.int64)
    percore = []
    for c in range(NCORES):
        sel = c_e == c
        cc = cell[sel]
        order = np.argsort(cc, kind="stable")
        counts[c] = np.bincount(cc, minlength=NCELL)
        percore.append((cc[order], lr[sel][order], colw[sel][order]))

    BPC = int(-(-counts.max() // P))  # uniform blocks per cell
    TOTB = NCELL * BPC

    gi_list, colv_list = [], []
    for c in range(NCORES):
        cc, lrc, cwc = percore[c]
        cell_start = np.zeros(NCELL, np.int64)
        cell_start[1:] = np.cumsum(counts[c])[:-1]
        rank = np.arange(cc.shape[0]) - cell_start[cc]
        dest = cc * (BPC * P) + rank
        gi_arr = np.zeros(TOTB * P, np.int16)
        cv_arr = np.full(TOTB * P, SENTINEL, np.float32)
        gi_arr[dest] = lrc
        cv_arr[dest] = cwc
        gi_list.append(np.ascontiguousarray(gi_arr.reshape(-1, 16).T))
        colv_list.append(
            np.ascontiguousarray(
                cv_arr.reshape(TOTB, P).T.astype(ml_dtypes.bfloat16)
            )
        )

    # per-node dinv in several layouts (padded nodes get 1.0)
    dinvP_list, dinv2P_list, dinvrow_list = [], [], []
    for c in range(NCORES):
        dloc = np.ones(PPC, np.float32)
        dloc[:TPC] = dinv_n[c * TPC : (c + 1) * TPC]
        dinvrow_list.append(dloc[None, :].copy())
        dinvP_list.append(np.ascontiguousarray(dloc.reshape(NG, P).T))
        dinv2P_list.append(np.ascontiguousarray((dloc * dloc).reshape(NG, P).T))

    sched = {"BPC": BPC, "TOTB": TOTB}
    return sched, gi_list, colv_list, dinvP_list, dinv2P_list, dinvrow_list


def build_kernel(sched, stage=9, debug=None):
    BPC = sched["BPC"]
    TOTB = sched["TOTB"]
    BSC = NW * BPC          # blocks per (sg, chunk) gather call
    NIDX = BSC * P          # idxs per gather call

    nc = bacc.Bacc("TRN2", target_bir_lowering=False, num_devices=NCORES)
    xTb = nc.dram_tensor("xTb", [P, PPC], BF16, kind="ExternalInput")
    gi = nc.dram_tensor("gi", [16, TOTB * 8], I16, kind="ExternalInput")
    colv = nc.dram_tensor("colv", [P, TOTB], BF16, kind="ExternalInput")
    dinvP = nc.dram_tensor("dinvP", [P, NG], F32, kind="ExternalInput")
    dinv2P = nc.dram_tensor("dinv2P", [P, NG], F32, kind="ExternalInput")
    dinvrow = nc.dram_tensor("dinvrow", [1, PPC], F32, kind="ExternalInput")
    W1 = nc.dram_tensor("W1", [IN, HID], F32, kind="ExternalInput")
    W2 = nc.dram_tensor("W2", [HID, OUT], F32, kind="ExternalInput")
    b1c = nc.dram_tensor("b1c", [P, 1], F32, kind="ExternalInput")
    b2c = nc.dram_tensor("b2c", [P, 1], F32, kind="ExternalInput")
    iota = nc.dram_tensor("iota", [P, W], BF16, kind="ExternalInput")
    iotaPc = nc.dram_tensor("iotaPc", [P, 2], F32, kind="ExternalInput")
    y = nc.dram_tensor("y", [OUT, PPC], F32, kind="ExternalOutput")
    dbg = None
    if debug is not None:
        dbg = nc.dram_tensor("dbg", [PPC, HID], F32, kind="ExternalOutput")

    t1q = nc.dram_tensor("t1q", [PPC, HID], BF16)
    t2q = nc.dram_tensor("t2q", [PPC, OUT], F32)
    table1 = nc.dram_tensor("table1", [NCH, CHROWS, HID], BF16, addr_space="Shared")
    table2 = nc.dram_tensor("table2", [NCH, CHROWS, OUT], F32, addr_space="Shared")

    t1qr = t1q.rearrange("(n p) f -> p n f", p=P)
    t2qr = t2q.rearrange("(n p) f -> p n f", p=P)

    with tile.TileContext(nc) as tc:
        with (
            tc.tile_pool(name="const", bufs=1) as cpool,
            tc.tile_pool(name="sb", bufs=2) as sb,
            tc.tile_pool(name="ps", bufs=1, space="PSUM") as psp,
        ):
            # ---- constants ----
            W1b = cpool.tile([IN, HID], BF16)
            nc.gpsimd.dma_start(out=W1b[:], in_=W1[:])
            W2b = cpool.tile([HID, OUT], BF16)
            nc.gpsimd.dma_start(out=W2b[:], in_=W2[:])
            b1t = cpool.tile([P, 1], F32)
            nc.sync.dma_start(out=b1t[:], in_=b1c[:])
            b2t = cpool.tile([P, 1], F32)
            nc.sync.dma_start(out=b2t[:], in_=b2c[:])
            iota_t = cpool.tile([P, W], BF16)
            nc.sync.dma_start(out=iota_t[:], in_=iota[:])
            iotaP_t = cpool.tile([P, 2], F32)
            nc.sync.dma_start(out=iotaP_t[:], in_=iotaPc[:])
            dinvP_t = cpool.tile([P, NG], F32)
            nc.sync.dma_start(out=dinvP_t[:], in_=dinvP[:])
            dinv2P_t = cpool.tile([P, NG], F32)
            nc.sync.dma_start(out=dinv2P_t[:], in_=dinv2P[:])
            gi_t = cpool.tile([P, TOTB * 8], I16)
            for r in range(8):
                nc.sync.dma_start(out=gi_t[16 * r : 16 * (r + 1), :], in_=gi[:])
            Sd01 = cpool.tile([P, 2, W], BF16)
            for half in range(2):
                nc.vector.tensor_scalar(
                    out=Sd01[:, half, :],
                    in0=iota_t[:],
                    scalar1=iotaP_t[:, half : half + 1],
                    scalar2=None,
                    op0=ALU.is_equal,
                )

            # ---- phase A: table1 rows = dinv[s] * (x @ W1), bf16 ----
            if stage >= 1:
                with tc.For_i(0, NSG, 1) as s:
                    xs = sb.tile([P, SEG], BF16, tag="msgs", name="xs")
                    nc.sync.dma_start(out=xs[:], in_=xTb[:, ds(s * SEG, SEG)])
                    dseg = sb.tile([P, SGG], F32, tag="dseg", name="dseg")
                    nc.sync.dma_start(out=dseg[:], in_=dinvP[:, ds(s * SGG, SGG)])
                    psA = psp.tile([P, SGG * P], F32, tag="psT", name="psA")
                    for gl in range(SGG):
                        nc.tensor.matmul(
                            out=psA[:, gl * P : (gl + 1) * P],
                            lhsT=xs[:, gl * P : (gl + 1) * P],
                            rhs=W1b[:],
                            start=True,
                            stop=True,
                        )
                    t1s = sb.tile([P, SGG, HID], BF16, tag="slab", name="t1s")
                    for gl in range(SGG):
                        nc.scalar.activation(
                            t1s[:, gl, :],
                            psA[:, gl * P : (gl + 1) * P],
                            AF.Copy,
                            scale=dseg[:, gl : gl + 1],
                        )
                    nc.sync.dma_start(
                        out=t1qr[:, ds(s * SGG, SGG), :], in_=t1s[:]
                    )
            if debug == "t1q":
                dq = cpool.tile([P, NG, HID], F32)
                nc.gpsimd.dma_start(out=dq[:], in_=t1qr[:])
                nc.sync.dma_start(
                    out=dbg.rearrange("(n p) f -> p n f", p=P)[:], in_=dq[:]
                )
            if stage >= 2:
                for s in range(NSG):
                    nc.gpsimd.collective_compute(
                        "AllGather",
                        ALU.bypass,
                        ins=[t1q[s * SEG : (s + 1) * SEG, :]],
                        outs=[table1[s]],
                        replica_groups=[list(range(NCORES))],
                    )

            def agg_layer(tablex, feat, msg_dtype, out_parts, lhs_slab, epilogue):
                """One aggregation layer inside a For_i over supergroups."""
                with tc.For_i(0, NSG, 1) as s:
                    dinvT = sb.tile([P, SEG], F32, tag="dinvT", name="dinvT", bufs=1)
                    nc.sync.dma_start(
                        out=dinvT[:],
                        in_=dinvrow[0:1, ds(s * SEG, SEG)].to_broadcast([P, SEG]),
                    )
                    psT = psp.tile([P, SEG], F32, tag="psT", name="psT")
                    nc.vector.memset(psT[:], 0.0)
                    for ch in range(NCH):
                        git = sb.tile([P, BSC * 8], I16, tag="git", name="git")
                        nc.sync.dma_start(
                            out=git[:],
                            in_=gi_t[:, ds((s * NCH + ch) * BSC * 8, BSC * 8)],
                        )
                        msgs = sb.tile([P, BSC, feat], msg_dtype, tag="msgs", name="msgs")
                        nc.gpsimd.dma_gather(
                            msgs[:],
                            tablex[ch],
                            git[:],
                            NIDX,
                            NIDX,
                            feat,
                            single_packet=False,
                        )
                        if msg_dtype != BF16:
                            mb = sb.tile([P, BSC, feat], BF16, tag="m2b", name="m2b")
                            nc.vector.tensor_copy(out=mb[:], in_=msgs[:])
                        else:
                            mb = msgs
                        cv = sb.tile([P, BSC], BF16, tag="cv", name="cv")
                        nc.sync.dma_start(
                            out=cv[:], in_=colv[:, ds((s * NCH + ch) * BSC, BSC)]
                        )
                        for w in range(NW):
                            S = sb.tile([P, BPC, W], BF16, tag="S", name="S")
                            nc.vector.tensor_tensor(
                                out=S[:],
                                in0=cv[:, w * BPC : (w + 1) * BPC, None].to_broadcast(
                                    [P, BPC, W]
                                ),
                                in1=iota_t[:, None, :].to_broadcast([P, BPC, W]),
                                op=ALU.is_equal,
                            )
                            for k in range(BPC):
                                nc.tensor.matmul(
                                    out=psT[0:out_parts, w * W : (w + 1) * W],
                                    lhsT=mb[:, w * BPC + k, :],
                                    rhs=S[:, k, :],
                                    start=False,
                                    stop=False,
                                    skip_group_check=True,
                                )
                    # self-loops: psT[:, gl-half] += lhsT=slab_gl, rhs=diag(dinv^2)
                    slab = lhs_slab(s)
                    for gl in range(SGG):
                        nc.tensor.matmul(
                            out=psT[0:out_parts, (gl // 2) * W : (gl // 2 + 1) * W],
                            lhsT=slab[:, gl, :],
                            rhs=Sd01[:, gl % 2, :],
                            start=False,
                            stop=(gl % 2 == 1),
                            skip_group_check=True,
                        )
                    epilogue(s, psT, dinvT)

            # ---- L1 ----
            def lhs_slab1(s):
                t1slab = sb.tile([P, SGG, HID], BF16, tag="slab", name="t1slab")
                nc.sync.dma_start(out=t1slab[:], in_=t1qr[:, ds(s * SGG, SGG), :])
                return t1slab

            def epi1(s, psT, dinvT):
                tmpT = sb.tile([P, SEG], F32, tag="tmpT", name="tmpT", bufs=1)
                nc.vector.tensor_tensor(
                    out=tmpT[:], in0=psT[:], in1=dinvT[:], op=ALU.mult
                )
                y1T = sb.tile([P, SEG], F32, tag="y1T", name="y1T", bufs=1)
                nc.vector.tensor_scalar(
                    out=y1T[:],
                    in0=tmpT[:],
                    scalar1=b1t[:],
                    scalar2=0.0,
                    op0=ALU.add,
                    op1=ALU.max,
                )
                t2preT = sb.tile([P, SEG], BF16, tag="t2preT", name="t2preT", bufs=1)
                nc.vector.tensor_tensor(
                    out=t2preT[:], in0=y1T[:], in1=dinvT[:], op=ALU.mult
                )
                ps2 = psp.tile([P, SGG * OUT], F32, tag="ps2", name="ps2")
                for gl in range(SGG):
                    nc.tensor.matmul(
                        out=ps2[:, gl * OUT : (gl + 1) * OUT],
                        lhsT=t2preT[:, gl * P : (gl + 1) * P],
                        rhs=W2b[:],
                        start=True,
                        stop=True,
                    )
                t2s = sb.tile([P, SGG, OUT], F32, tag="t2s", name="t2s")
                nc.scalar.activation(t2s[:], ps2[:], AF.Copy)
                nc.sync.dma_start(out=t2qr[:, ds(s * SGG, SGG), :], in_=t2s[:])

            if stage >= 3:
                agg_layer(table1, HID, BF16, P, lhs_slab1, epi1)

            if debug == "t2q":
                dq2 = cpool.tile([P, NG, OUT], F32)
                nc.sync.dma_start(out=dq2[:], in_=t2qr[:])
                nc.sync.dma_start(
                    out=dbg.rearrange("(n p) f -> p n f", p=P)[:, :, :OUT], in_=dq2[:]
                )
            if debug == "dinvT":
                dv = sb.tile([P, SEG], F32, tag="dinvT", name="dvdump", bufs=1)
                nc.sync.dma_start(
                    out=dv[:], in_=dinvrow[0:1, 0:SEG].to_broadcast([P, SEG])
                )
                nc.sync.dma_start(
                    out=dbg[0:SEG, 0:P].rearrange("a b -> b a"), in_=dv[:]
                )
            if stage >= 4:
                for s in range(NSG):
                    nc.gpsimd.collective_compute(
                        "AllGather",
                        ALU.bypass,
                        ins=[t2q[s * SEG : (s + 1) * SEG, :]],
                        outs=[table2[s]],
                        replica_groups=[list(range(NCORES))],
                    )

            # ---- L2 ----
            def lhs_slab2(s):
                t2slabf = sb.tile([P, SGG, OUT], F32, tag="t2s", name="t2slabf")
                nc.sync.dma_start(out=t2slabf[:], in_=t2qr[:, ds(s * SGG, SGG), :])
                t2slab = sb.tile([P, SGG, OUT], BF16, tag="slab", name="t2slab")
                nc.vector.tensor_copy(out=t2slab[:], in_=t2slabf[:])
                return t2slab

            def epi2(s, psT, dinvT):
                outT = sb.tile([OUT, SEG], F32, tag="outT", name="outT", bufs=1)
                nc.vector.tensor_tensor(
                    out=outT[:], in0=psT[0:OUT, :], in1=dinvT[0:OUT, :], op=ALU.mult
                )
                outT2 = sb.tile([OUT, SEG], F32, tag="outT2", name="outT2", bufs=1)
                nc.vector.tensor_scalar(
                    out=outT2[:],
                    in0=outT[:],
                    scalar1=b2t[0:OUT, :],
                    scalar2=None,
                    op0=ALU.add,
                )
                nc.sync.dma_start(out=y[:, ds(s * SEG, SEG)], in_=outT2[:])

            if stage >= 5:
                agg_layer(table2, OUT, F32, OUT, lhs_slab2, epi2)

    nc.finalize()
    return nc


def make_in_maps(inputs, sched, gi_list, colv_list, dinvP_list, dinv2P_list, dinvrow_list):
    x = np.asarray(inputs["x"], np.float32)
    W1 = np.asarray(inputs["W1"], np.float32)
    W2 = np.asarray(inputs["W2"], np.float32)
    b1 = np.asarray(inputs["b1"], np.float32)
    b2 = np.asarray(inputs["b2"], np.float32)
    iota_np = np.tile(np.arange(W, dtype=ml_dtypes.bfloat16)[None, :], (P, 1))
    iotaPc_np = np.zeros((P, 2), np.float32)
    iotaPc_np[:, 0] = np.arange(P, dtype=np.float32)
    iotaPc_np[:, 1] = (np.arange(P) + 128).astype(np.float32)
    b1col = b1[:, None].astype(np.float32)
    b2col = np.zeros((P, 1), np.float32)
    b2col[:OUT, 0] = b2
    in_maps = []
    for c in range(NCORES):
        xs = np.zeros((P, PPC), ml_dtypes.bfloat16)
        xs[:, :TPC] = x[c * TPC : (c + 1) * TPC].T.astype(ml_dtypes.bfloat16)
        in_maps.append(
            {
                "xTb": xs,
                "gi": gi_list[c],
                "colv": colv_list[c],
                "dinvP": dinvP_list[c],
                "dinv2P": dinv2P_list[c],
                "dinvrow": dinvrow_list[c],
                "W1": W1,
                "W2": W2,
                "b1c": b1col,
                "b2c": b2col,
                "iota": iota_np,
                "iotaPc": iotaPc_np,
            }
        )
    return in_maps


def assemble_output(results):
    outs = []
    for c in range(NCORES):
        yc = results[c]["y"]  # [OUT, PPC]
        outs.append(yc[:, :TPC].T)
    return np.ascontiguousarray(np.concatenate(outs, axis=0))


def kernel(**inputs):
    prep = host_prep(inputs["edge_index"])
    sched = prep[0]
    nc = build_kernel(sched)
    in_maps = make_in_maps(inputs, *prep)
    res = run_bass_kernel_spmd(nc, in_maps, core_ids=list(range(NCORES)))
    return assemble_output(res.results)
